# revision 16
# baseline (speedup 1.0000x reference)
"""Multi-head GQA attention prefill (B=1, S=2048, D=4096, 32 q-heads /
8 kv-heads, head_dim=128, RoPE, causal) on 8 TRN2 NeuronCores.

Sharding: tensor-parallel over heads. Core c owns q-heads [4c, 4c+4) and
kv-head c (the GQA group boundary coincides with the core boundary, so
attention is fully local). The out-projection is sharded over wo ROWS
(output columns): after attention each core AllGathers the (transposed,
normalized) attention outputs of all heads and computes its 512 output
columns; the host concatenates the 8 column slices.

Compute dtype: bf16 matmul operands with fp32 PSUM accumulation; softmax
statistics in fp32.  All matmuls run in the transposed "P^T" dataflow:
  qT/kT  [head_dim, S]  = proj(xT)           (RoPE'd in-place on DVE)
  S^T    [Sk, Sq]       = kT_chunk.T @ qT    (causal blocks only)
  expS   bf16           = exp(S^T / sqrt(d)) (ScalarE, PSUM->SBUF)
  oT     [head_dim, Sq] = sum_k V_chunk.T @ expS   (V from PE-transpose)
  rowsum [1, Sq]        = ones.T @ expS      (fp32 via PSUM accumulate)
so no transposes are needed anywhere else in the chain.

RoPE trick: attention scores are invariant under a fixed permutation of
each head's 128 dims applied to BOTH q and k, so the host deinterleaves
wq/wk rows to [evens; odds].  RoPE then becomes two partition-halves
ops: new_e = e*cos - o*sin, new_o = e*sin + o*cos with [64, S] tiles.
"""

import sys

sys.path.insert(0, "/opt/trn_rl_repo")

import numpy as np
import ml_dtypes

import concourse.bass as bass
import concourse.mybir as mybir
from concourse import bacc, tile
from concourse.bass_utils import run_bass_kernel_spmd
from concourse.masks import make_identity

F32 = mybir.dt.float32
BF16 = mybir.dt.bfloat16
BF16_NP = ml_dtypes.bfloat16

NCORES = 8
S = 2048
D = 4096
HD = 128                 # head dim
QH = 4                   # q heads per core
QROWS = QH * HD          # 512 q rows per core
SB = 512                 # seq superblock (free dim of most matmuls)
NSB = S // SB            # 4
DC = D // 128            # 32 contraction chunks
NKC = S // 128           # 16 key chunks
SCALE = 1.0 / np.sqrt(HD)


def build_graph():
    nc = bacc.Bacc("TRN2", target_bir_lowering=False, debug=False,
                   num_devices=NCORES)

    xT = nc.declare_dram_parameter("xT", [D, S], BF16, isOutput=False)
    wqT = nc.declare_dram_parameter("wqT", [D, QROWS], BF16, isOutput=False)
    wkT = nc.declare_dram_parameter("wkT", [D, HD], BF16, isOutput=False)
    wvT = nc.declare_dram_parameter("wvT", [D, HD], BF16, isOutput=False)
    woT = nc.declare_dram_parameter("woT", [D, SB], BF16, isOutput=False)
    cosT = nc.declare_dram_parameter("cosT", [64, S], F32, isOutput=False)
    sinT = nc.declare_dram_parameter("sinT", [64, S], F32, isOutput=False)
    mask = nc.declare_dram_parameter("mask", [128, 128], BF16, isOutput=False)
    out = nc.declare_dram_parameter("out", [S, SB], F32, isOutput=True)

    aT_loc = [nc.dram_tensor(f"aT_loc{sb}", [QROWS, SB], BF16)
              for sb in range(NSB)]
    aT_all = [nc.dram_tensor(f"aT_all{sb}", [NCORES * QROWS, SB], BF16,
                             addr_space="Shared") for sb in range(NSB)]

    with tile.TileContext(nc) as tc:
        with tc.tile_pool(name="const", bufs=1) as cpool, \
             tc.tile_pool(name="wts", bufs=1) as wpool, \
             tc.tile_pool(name="qkv", bufs=1) as qkvpool, \
             tc.tile_pool(name="xs", bufs=4) as xpool, \
             tc.tile_pool(name="rope", bufs=2) as rpool, \
             tc.tile_pool(name="exps", bufs=4) as epool, \
             tc.tile_pool(name="onorm", bufs=2) as opool, \
             tc.tile_pool(name="ostream", bufs=3) as spool, \
             tc.tile_pool(name="ps", bufs=8, space="PSUM") as ps:

            # ---- weights: chunked DMAs so the first matmul starts early ----
            wq_sb = wpool.tile([128, DC, QROWS], BF16, tag="wq")
            wk_sb = wpool.tile([128, DC, HD], BF16, tag="wk")
            wv_sb = wpool.tile([128, DC, HD], BF16, tag="wv")
            wo_sb = wpool.tile([128, DC, SB], BF16, tag="wo")
            wqr = wqT.rearrange("(c p) m -> p c m", p=128)
            wkr = wkT.rearrange("(c p) m -> p c m", p=128)
            wvr = wvT.rearrange("(c p) m -> p c m", p=128)
            wor = woT.rearrange("(c p) m -> p c m", p=128)
            G = 8
            for g in range(0, DC, G):
                s = slice(g, g + G)
                nc.gpsimd.dma_start(wk_sb[:, s, :], wkr[:, s, :])
                nc.gpsimd.dma_start(wv_sb[:, s, :], wvr[:, s, :])
                nc.gpsimd.dma_start(wq_sb[:, s, :], wqr[:, s, :])

            # ---- constants ----
            cos1 = cpool.tile([64, S], F32, tag="cos1")
            nc.gpsimd.dma_start(cos1[:], cosT[:, :])
            sin1 = cpool.tile([64, S], F32, tag="sin1")
            nc.gpsimd.dma_start(sin1[:], sinT[:, :])
            mask_t = cpool.tile([128, 128], BF16, tag="mask")
            nc.gpsimd.dma_start(mask_t[:], mask[:])
            ident = cpool.tile([128, 128], BF16, tag="ident")
            make_identity(nc, ident[:])
            ones_col = cpool.tile([128, 1], F32, tag="ones_col")
            nc.vector.memset(ones_col[:], 1.0)
            ones_row = cpool.tile([1, 128], F32, tag="ones_row")
            nc.vector.memset(ones_row[:], 1.0)
            for g in range(0, DC, G):
                nc.gpsimd.dma_start(wo_sb[:, g:g + G, :], wor[:, g:g + G, :])

            # ---- persistent activations ----
            qT = [qkvpool.tile([128, S], BF16, tag=f"qT{h}", name=f"qT{h}")
                  for h in range(QH)]
            kT = qkvpool.tile([128, S], BF16, tag="kT")
            v_sb = [qkvpool.tile([128, HD], BF16, tag=f"v{kc}", name=f"v{kc}")
                    for kc in range(NKC)]

            aus = {}     # (sb, h) -> unnormalized oT tile
            sums = {}    # sb -> rowsum collection tile
            acols = {}   # (sb, j) -> prefetched gathered-aT column block

            def alloc_proj(sb):
                pq = [ps.tile([128, SB], F32, tag="ps", name=f"pq{sb}_{h}")
                      for h in range(QH)]
                pk = ps.tile([128, SB], F32, tag="ps", name=f"pk{sb}")
                pv = ps.tile([128, SB], F32, tag="ps", name=f"pv{sb}")
                return pq, pk, pv

            def proj_steps(sb, tiles):
                """Two 3-bank passes so a concurrent attention block fits in
                PSUM; yields after every 3 matmuls (one dc of one pass)."""
                pq, pk, pv = tiles
                cols = bass.ts(sb, SB)
                passes = [
                    [(pk, wk_sb, None), (pv, wv_sb, None), (pq[0], wq_sb, 0)],
                    [(pq[1], wq_sb, 1), (pq[2], wq_sb, 2), (pq[3], wq_sb, 3)],
                ]
                for pas in passes:
                    for dc in range(DC):
                        xt = xpool.tile([128, SB], BF16, tag="xt")
                        nc.sync.dma_start(xt[:], xT[bass.ts(dc, 128), cols])
                        st, sp = dc == 0, dc == DC - 1
                        for (pt_, wt, h) in pas:
                            w = wt[:, dc, :] if h is None else \
                                wt[:, dc, bass.ts(h, HD)]
                            nc.tensor.matmul(pt_[:], w, xt[:],
                                             start=st, stop=sp)
                        yield

            def emit_proj(sb, tiles=None):
                tiles = tiles or alloc_proj(sb)
                for _ in proj_steps(sb, tiles):
                    pass
                return tiles

            def rope(psum, dst, cols):
                # dst[e] = p[e]*cos - p[o]*sin ; dst[o] = p[o]*cos + p[e]*sin
                # tb holds the sin-products half-SWAPPED so that the final
                # add/sub reads same-base-partition SBUF operands.
                ta = rpool.tile([128, SB], F32, tag="rope_a")
                tb = rpool.tile([128, SB], F32, tag="rope_b")
                nc.vector.tensor_mul(ta[0:64, :], psum[0:64, :], cos1[:, cols])
                nc.vector.tensor_mul(ta[64:128, :], psum[64:128, :],
                                     cos1[:, cols])
                nc.vector.tensor_mul(tb[0:64, :], psum[64:128, :],
                                     sin1[:, cols])
                nc.vector.tensor_mul(tb[64:128, :], psum[0:64, :],
                                     sin1[:, cols])
                nc.vector.tensor_sub(dst[0:64, cols], ta[0:64, :], tb[0:64, :])
                nc.vector.tensor_add(dst[64:128, cols], ta[64:128, :],
                                     tb[64:128, :])

            def emit_rope_v(sb, tiles):
                pq, pk, pv = tiles
                cols = bass.ts(sb, SB)
                rope(pk, kT, cols)          # k first: attention h=0 needs it
                rope(pq[0], qT[0], cols)
                vt = rpool.tile([128, SB], BF16, tag="vt")
                nc.vector.tensor_copy(vt[:], pv[:])
                for j in range(SB // 128):
                    pt = ps.tile([128, 128], BF16, tag="ps", name="pt")
                    nc.tensor.transpose(pt[:], vt[:, bass.ts(j, 128)], ident[:])
                    nc.vector.tensor_copy(v_sb[4 * sb + j][:], pt[:])
                for h in range(1, QH):
                    rope(pq[h], qT[h], cols)

            def emit_attn(sb, filler=None):
                def fill():
                    if filler is not None:
                        next(filler, None)
                nkc = 4 * sb + 4
                sm = opool.tile([128, SB], F32, tag="sums", bufs=2,
                                name=f"sums{sb}")
                nc.gpsimd.memset(sm[:], 1.0)
                sums[sb] = sm
                for h in range(QH):
                    po = ps.tile([128, SB], F32, tag="ps", name="po")
                    acc = epool.tile([128, SB], F32, tag="acc", bufs=2)
                    es = {}

                    def qk(kc):
                        # diagonal blocks: columns < 128j are fully masked --
                        # skip them in both the matmul and the exp.
                        j = kc - 4 * sb
                        c0 = 128 * j if j > 0 else 0
                        pg = ps.tile([128, SB], F32, tag="ps", name="pg")
                        nc.tensor.matmul(
                            pg[:, c0:SB], kT[:, bass.ts(kc, 128)],
                            qT[h][:, sb * SB + c0:(sb + 1) * SB],
                            start=True, stop=True)
                        e = epool.tile([128, SB], BF16, tag="es")
                        nc.scalar.activation(e[:, c0:SB], pg[:, c0:SB],
                                             mybir.ActivationFunctionType.Exp,
                                             scale=SCALE)
                        if c0 > 0:
                            nc.vector.memset(e[:, 0:c0], 0.0)
                        if j >= 0:
                            nc.vector.tensor_mul(e[:, bass.ts(j, 128)],
                                                 e[:, bass.ts(j, 128)], mask_t[:])
                        es[kc] = e

                    def pv_ones(kc):
                        st, sp = kc == 0, kc == nkc - 1
                        nc.tensor.matmul(po[:], v_sb[kc][:], es[kc][:],
                                         start=st, stop=sp)
                        if st:
                            nc.vector.tensor_copy(acc[:], es[kc][:])
                        else:
                            nc.vector.tensor_add(acc[:], acc[:], es[kc][:])
                        del es[kc]

                    qk(0)
                    for kc in range(1, nkc):
                        qk(kc)
                        pv_ones(kc - 1)
                        fill()
                    pv_ones(nkc - 1)
                    fill()

                    psum = ps.tile([1, SB], F32, tag="ps", name="psum")
                    nc.tensor.matmul(psum[:], ones_col[:], acc[:],
                                     start=True, stop=True)
                    nc.vector.tensor_copy(sm[32 * h:32 * h + 1, :], psum[:])
                    au = opool.tile([128, SB], BF16, tag="au", bufs=8,
                                    name=f"au{sb}_{h}")
                    nc.vector.tensor_copy(au[:], po[:])
                    aus[(sb, h)] = au

            def emit_norm_ag(sb):
                rec = opool.tile([128, SB], F32, tag="rec", bufs=2)
                nc.vector.reciprocal(rec[:], sums[sb][:])
                for h in range(QH):
                    rc = opool.tile([1, SB], F32, tag="rc", bufs=2)
                    nc.gpsimd.tensor_copy(rc[:], rec[32 * h:32 * h + 1, :])
                    pb = ps.tile([128, SB], F32, tag="ps", name="pb")
                    nc.tensor.matmul(pb[:], ones_row[:], rc[:],
                                     start=True, stop=True)
                    at = opool.tile([128, SB], BF16, tag="at")
                    nc.vector.tensor_mul(at[:], aus[(sb, h)][:], pb[:])
                    nc.gpsimd.dma_start(aT_loc[sb][bass.ts(h, 128), :], at[:])
                    del aus[(sb, h)]
                nc.gpsimd.collective_compute(
                    "AllGather",
                    mybir.AluOpType.bypass,
                    ins=[aT_loc[sb][:]],
                    outs=[aT_all[sb][:]],
                    replica_groups=[list(range(NCORES))],
                )
                for j in range(4):
                    acol = spool.tile([128, DC, 128], BF16, tag="acol",
                                      bufs=4, name=f"acol{sb}_{j}")
                    nc.gpsimd.dma_start(
                        acol[:],
                        aT_all[sb][:, bass.ts(j, 128)].rearrange(
                            "(c p) m -> p c m", p=128))
                    acols[(sb, j)] = acol

            def outproj_steps(sbp, group=4):
                """Generator: each next() emits `group` out-proj matmuls."""
                for j in range(4):
                    mc = 4 * sbp + j
                    acol = acols.pop((sbp, j))
                    pout = ps.tile([128, SB], F32, tag="ps", name="pout")
                    for hc0 in range(0, DC, group):
                        for hc in range(hc0, min(hc0 + group, DC)):
                            nc.tensor.matmul(pout[:], acol[:, hc, :],
                                             wo_sb[:, hc, :],
                                             start=hc == 0, stop=hc == DC - 1)
                        yield
                    ot = spool.tile([128, SB], F32, tag="ot")
                    nc.vector.tensor_copy(ot[:], pout[:])
                    nc.sync.dma_start(out[bass.ts(mc, 128), :], ot[:])

            def emit_outproj(sbp):
                for _ in outproj_steps(sbp):
                    pass

            # ---- schedule ----
            # PE stream: proj(0) rope(0) attn(0) | proj(1) norm(0)+AG(0)
            # rope(1) attn(1) | proj(2) norm(1)+AG(1) rope(2)+outproj(0)
            # attn(2) | proj(3) norm(2)+AG(2) rope(3)+outproj(1) attn(3) |
            # norm(3)+AG(3) outproj(2) outproj(3)
            def drain(gen):
                for _ in gen:
                    pass

            t0_ = emit_proj(0)
            emit_rope_v(0, t0_)
            t1_ = alloc_proj(1)
            g1 = proj_steps(1, t1_)
            emit_attn(0, g1)          # proj(1) interleaves into attn(0)
            drain(g1)
            emit_norm_ag(0)
            emit_rope_v(1, t1_)
            t2_ = alloc_proj(2)
            g2 = proj_steps(2, t2_)
            emit_attn(1, g2)          # proj(2) pass1 interleaves into attn(1)
            drain(g2)
            emit_norm_ag(1)
            emit_rope_v(2, t2_)
            go0 = outproj_steps(0, group=3)
            emit_attn(2, go0)         # outproj(0) rides inside attn(2)
            drain(go0)
            emit_norm_ag(2)
            t3_ = emit_proj(3)
            emit_rope_v(3, t3_)
            go1 = outproj_steps(1, group=1)
            emit_attn(3, go1)         # half of outproj(1) rides inside attn(3)
            emit_norm_ag(3)
            drain(go1)                # rest covers the AG(3) window
            emit_outproj(NSB - 2)
            emit_outproj(NSB - 1)

    nc.compile()
    return nc


_PERM = np.concatenate([np.arange(0, HD, 2), np.arange(1, HD, 2)])


def _prep_inputs(x, wq, wk, wv, wo, freqs_cos, freqs_sin):
    xT = np.ascontiguousarray(x.reshape(S, D).T).astype(BF16_NP)
    cosT = np.ascontiguousarray(freqs_cos.T).astype(np.float32)
    sinT = np.ascontiguousarray(freqs_sin.T).astype(np.float32)
    mask = np.triu(np.ones((128, 128), dtype=np.float32)).astype(BF16_NP)

    qperm = np.concatenate([h * HD + _PERM for h in range(QH)])
    in_maps = []
    for c in range(NCORES):
        wq_c = wq[c * QROWS:(c + 1) * QROWS][qperm]
        wk_c = wk[c * HD:(c + 1) * HD][_PERM]
        wv_c = wv[c * HD:(c + 1) * HD]
        wo_c = wo[c * SB:(c + 1) * SB]
        in_maps.append({
            "xT": xT,
            "wqT": np.ascontiguousarray(wq_c.T).astype(BF16_NP),
            "wkT": np.ascontiguousarray(wk_c.T).astype(BF16_NP),
            "wvT": np.ascontiguousarray(wv_c.T).astype(BF16_NP),
            "woT": np.ascontiguousarray(wo_c.T).astype(BF16_NP),
            "cosT": cosT,
            "sinT": sinT,
            "mask": mask,
        })
    return in_maps


def kernel(x, wq, wk, wv, wo, freqs_cos, freqs_sin, start_pos=0, *,
           _trace=False):
    x = np.asarray(x, dtype=np.float32)
    in_maps = _prep_inputs(np.asarray(x, np.float32), np.asarray(wq, np.float32),
                           np.asarray(wk, np.float32), np.asarray(wv, np.float32),
                           np.asarray(wo, np.float32),
                           np.asarray(freqs_cos, np.float32),
                           np.asarray(freqs_sin, np.float32))
    nc = build_graph()
    res = run_bass_kernel_spmd(nc, in_maps, core_ids=list(range(NCORES)),
                               trace=_trace)
    full = np.concatenate([res.results[c]["out"] for c in range(NCORES)],
                          axis=1)
    out = full.reshape(1, S, D).astype(np.float32)
    if _trace:
        return out, res
    return out


# revision 17
# speedup vs baseline: 1.2040x; 1.2040x over previous
"""Multi-head GQA attention prefill (B=1, S=2048, D=4096, 32 q-heads /
8 kv-heads, head_dim=128, RoPE, causal) on 8 TRN2 NeuronCores.

Sharding: tensor-parallel over heads. Core c owns q-heads [4c, 4c+4) and
kv-head c (the GQA group boundary coincides with the core boundary, so
attention is fully local). The out-projection is sharded over wo ROWS
(output columns): after attention each core AllGathers the (transposed,
normalized) attention outputs of all heads and computes its 512 output
columns; the host concatenates the 8 column slices.

Compute dtype: bf16 matmul operands with fp32 PSUM accumulation; softmax
statistics in fp32.  All matmuls run in the transposed "P^T" dataflow:
  qT/kT  [head_dim, S]  = proj(xT)           (RoPE'd in-place on DVE)
  S^T    [Sk, Sq]       = kT_chunk.T @ qT    (causal blocks only)
  expS   bf16           = exp(S^T / sqrt(d)) (ScalarE, PSUM->SBUF)
  oT     [head_dim, Sq] = sum_k V_chunk.T @ expS   (V from PE-transpose)
  rowsum [1, Sq]        = ones.T @ expS      (fp32 via PSUM accumulate)
so no transposes are needed anywhere else in the chain.

RoPE trick: attention scores are invariant under a fixed permutation of
each head's 128 dims applied to BOTH q and k, so the host deinterleaves
wq/wk rows to [evens; odds].  RoPE then becomes two partition-halves
ops: new_e = e*cos - o*sin, new_o = e*sin + o*cos with [64, S] tiles.
"""

import sys

sys.path.insert(0, "/opt/trn_rl_repo")

import numpy as np
import ml_dtypes

import concourse.bass as bass
import concourse.mybir as mybir
from concourse import bacc, tile
from concourse.bass_utils import run_bass_kernel_spmd
from concourse.masks import make_identity

F32 = mybir.dt.float32
BF16 = mybir.dt.bfloat16
BF16_NP = ml_dtypes.bfloat16

NCORES = 8
S = 2048
D = 4096
HD = 128                 # head dim
QH = 4                   # q heads per core
QROWS = QH * HD          # 512 q rows per core
SB = 512                 # seq superblock (free dim of most matmuls)
NSB = S // SB            # 4
DC = D // 128            # 32 contraction chunks
NKC = S // 128           # 16 key chunks
SCALE = 1.0 / np.sqrt(HD)


def build_graph():
    nc = bacc.Bacc("TRN2", target_bir_lowering=False, debug=False,
                   num_devices=NCORES)

    xT = nc.declare_dram_parameter("xT", [D, S], BF16, isOutput=False)
    wqT = nc.declare_dram_parameter("wqT", [D, QROWS], BF16, isOutput=False)
    wkT = nc.declare_dram_parameter("wkT", [D, HD], BF16, isOutput=False)
    wvT = nc.declare_dram_parameter("wvT", [D, HD], BF16, isOutput=False)
    woT = nc.declare_dram_parameter("woT", [D, SB], BF16, isOutput=False)
    cosT = nc.declare_dram_parameter("cosT", [64, S], F32, isOutput=False)
    sinT = nc.declare_dram_parameter("sinT", [64, S], F32, isOutput=False)
    mask = nc.declare_dram_parameter("mask", [128, 128], BF16, isOutput=False)
    out = nc.declare_dram_parameter("out", [S, SB], F32, isOutput=True)

    aT_loc = [nc.dram_tensor(f"aT_loc{sb}", [QROWS, SB], BF16)
              for sb in range(NSB)]
    aT_all = [nc.dram_tensor(f"aT_all{sb}", [NCORES * QROWS, SB], BF16,
                             addr_space="Shared") for sb in range(NSB)]

    with tile.TileContext(nc) as tc:
        with tc.tile_pool(name="const", bufs=1) as cpool, \
             tc.tile_pool(name="wts", bufs=1) as wpool, \
             tc.tile_pool(name="qkv", bufs=1) as qkvpool, \
             tc.tile_pool(name="xs", bufs=4) as xpool, \
             tc.tile_pool(name="rope", bufs=2) as rpool, \
             tc.tile_pool(name="exps", bufs=4) as epool, \
             tc.tile_pool(name="onorm", bufs=2) as opool, \
             tc.tile_pool(name="ostream", bufs=3) as spool, \
             tc.tile_pool(name="ps", bufs=8, space="PSUM") as ps:

            # ---- weights: chunked DMAs so the first matmul starts early ----
            wq_sb = wpool.tile([128, DC, QROWS], BF16, tag="wq")
            wk_sb = wpool.tile([128, DC, HD], BF16, tag="wk")
            wv_sb = wpool.tile([128, DC, HD], BF16, tag="wv")
            wo_sb = wpool.tile([128, DC, SB], BF16, tag="wo")
            wqr = wqT.rearrange("(c p) m -> p c m", p=128)
            wkr = wkT.rearrange("(c p) m -> p c m", p=128)
            wvr = wvT.rearrange("(c p) m -> p c m", p=128)
            wor = woT.rearrange("(c p) m -> p c m", p=128)
            G = 8
            for g in range(0, DC, G):
                s = slice(g, g + G)
                nc.gpsimd.dma_start(wk_sb[:, s, :], wkr[:, s, :])
                nc.gpsimd.dma_start(wv_sb[:, s, :], wvr[:, s, :])
                nc.gpsimd.dma_start(wq_sb[:, s, :], wqr[:, s, :])

            # ---- constants ----
            cos1 = cpool.tile([64, S], F32, tag="cos1")
            nc.gpsimd.dma_start(cos1[:], cosT[:, :])
            sin1 = cpool.tile([64, S], F32, tag="sin1")
            nc.gpsimd.dma_start(sin1[:], sinT[:, :])
            mask_t = cpool.tile([128, 128], BF16, tag="mask")
            nc.gpsimd.dma_start(mask_t[:], mask[:])
            ident = cpool.tile([128, 128], BF16, tag="ident")
            make_identity(nc, ident[:])
            ones_col = cpool.tile([128, 1], F32, tag="ones_col")
            nc.vector.memset(ones_col[:], 1.0)
            ones_row = cpool.tile([1, 128], F32, tag="ones_row")
            nc.vector.memset(ones_row[:], 1.0)
            for g in range(0, DC, G):
                nc.gpsimd.dma_start(wo_sb[:, g:g + G, :], wor[:, g:g + G, :])

            # ---- persistent activations ----
            qT = [qkvpool.tile([128, S], BF16, tag=f"qT{h}", name=f"qT{h}")
                  for h in range(QH)]
            kT = qkvpool.tile([128, S], BF16, tag="kT")
            v_sb = [qkvpool.tile([128, HD], BF16, tag=f"v{kc}", name=f"v{kc}")
                    for kc in range(NKC)]

            aus = {}     # (sb, h) -> unnormalized oT tile
            sums = {}    # sb -> rowsum collection tile
            acols = {}   # (sb, j) -> prefetched gathered-aT column block

            def alloc_proj(sb):
                pq = [ps.tile([128, SB], F32, tag="ps", name=f"pq{sb}_{h}")
                      for h in range(QH)]
                pk = ps.tile([128, SB], F32, tag="ps", name=f"pk{sb}")
                pv = ps.tile([128, SB], F32, tag="ps", name=f"pv{sb}")
                return pq, pk, pv

            def proj_steps(sb, tiles):
                """Two 3-bank passes so a concurrent attention block fits in
                PSUM; yields after every 3 matmuls (one dc of one pass)."""
                pq, pk, pv = tiles
                cols = bass.ts(sb, SB)
                passes = [
                    [(pk, wk_sb, None), (pv, wv_sb, None), (pq[0], wq_sb, 0)],
                    [(pq[1], wq_sb, 1), (pq[2], wq_sb, 2), (pq[3], wq_sb, 3)],
                ]
                for pas in passes:
                    for dc in range(DC):
                        xt = xpool.tile([128, SB], BF16, tag="xt")
                        nc.sync.dma_start(xt[:], xT[bass.ts(dc, 128), cols])
                        st, sp = dc == 0, dc == DC - 1
                        for (pt_, wt, h) in pas:
                            w = wt[:, dc, :] if h is None else \
                                wt[:, dc, bass.ts(h, HD)]
                            nc.tensor.matmul(pt_[:], w, xt[:],
                                             start=st, stop=sp)
                        yield

            def emit_proj(sb, tiles=None):
                tiles = tiles or alloc_proj(sb)
                for _ in proj_steps(sb, tiles):
                    pass
                return tiles

            def rope(psum, dst, cols):
                # dst[e] = p[e]*cos - p[o]*sin ; dst[o] = p[o]*cos + p[e]*sin
                # tb holds the sin-products half-SWAPPED so that the final
                # add/sub reads same-base-partition SBUF operands.
                ta = rpool.tile([128, SB], F32, tag="rope_a")
                tb = rpool.tile([128, SB], F32, tag="rope_b")
                nc.vector.tensor_mul(ta[0:64, :], psum[0:64, :], cos1[:, cols])
                nc.vector.tensor_mul(ta[64:128, :], psum[64:128, :],
                                     cos1[:, cols])
                nc.vector.tensor_mul(tb[0:64, :], psum[64:128, :],
                                     sin1[:, cols])
                nc.vector.tensor_mul(tb[64:128, :], psum[0:64, :],
                                     sin1[:, cols])
                nc.vector.tensor_sub(dst[0:64, cols], ta[0:64, :], tb[0:64, :])
                nc.vector.tensor_add(dst[64:128, cols], ta[64:128, :],
                                     tb[64:128, :])

            def emit_rope_v(sb, tiles):
                pq, pk, pv = tiles
                cols = bass.ts(sb, SB)
                rope(pk, kT, cols)          # k first: attention h=0 needs it
                rope(pq[0], qT[0], cols)
                vt = rpool.tile([128, SB], BF16, tag="vt")
                nc.vector.tensor_copy(vt[:], pv[:])
                for j in range(SB // 128):
                    pt = ps.tile([128, 128], BF16, tag="ps", name="pt")
                    nc.tensor.transpose(pt[:], vt[:, bass.ts(j, 128)], ident[:])
                    nc.vector.tensor_copy(v_sb[4 * sb + j][:], pt[:])
                for h in range(1, QH):
                    rope(pq[h], qT[h], cols)

            def emit_attn(sb, filler=None):
                def fill():
                    if filler is not None:
                        next(filler, None)
                nkc = 4 * sb + 4
                sm = opool.tile([128, SB], F32, tag="sums", bufs=2,
                                name=f"sums{sb}")
                nc.gpsimd.memset(sm[:], 1.0)
                sums[sb] = sm
                for h in range(QH):
                    po = ps.tile([128, SB], F32, tag="ps", name="po")
                    acc = epool.tile([128, SB], F32, tag="acc", bufs=2)
                    es = {}

                    def qk(kc):
                        # diagonal blocks: columns < 128j are fully masked --
                        # skip them in both the matmul and the exp.
                        j = kc - 4 * sb
                        c0 = 128 * j if j > 0 else 0
                        pg = ps.tile([128, SB], F32, tag="ps", name="pg")
                        nc.tensor.matmul(
                            pg[:, c0:SB], kT[:, bass.ts(kc, 128)],
                            qT[h][:, sb * SB + c0:(sb + 1) * SB],
                            start=True, stop=True)
                        e = epool.tile([128, SB], BF16, tag="es")
                        nc.scalar.activation(e[:, c0:SB], pg[:, c0:SB],
                                             mybir.ActivationFunctionType.Exp,
                                             scale=SCALE)
                        if c0 > 0:
                            nc.vector.memset(e[:, 0:c0], 0.0)
                        if j >= 0:
                            nc.vector.tensor_mul(e[:, bass.ts(j, 128)],
                                                 e[:, bass.ts(j, 128)], mask_t[:])
                        es[kc] = e

                    def pv_ones(kc):
                        st, sp = kc == 0, kc == nkc - 1
                        nc.tensor.matmul(po[:], v_sb[kc][:], es[kc][:],
                                         start=st, stop=sp)
                        if st:
                            nc.vector.tensor_copy(acc[:], es[kc][:])
                        else:
                            nc.vector.tensor_add(acc[:], acc[:], es[kc][:])
                        del es[kc]

                    qk(0)
                    for kc in range(1, nkc):
                        qk(kc)
                        pv_ones(kc - 1)
                        fill()
                    pv_ones(nkc - 1)
                    fill()

                    psum = ps.tile([1, SB], F32, tag="ps", name="psum")
                    nc.tensor.matmul(psum[:], ones_col[:], acc[:],
                                     start=True, stop=True)
                    nc.vector.tensor_copy(sm[32 * h:32 * h + 1, :], psum[:])
                    au = opool.tile([128, SB], BF16, tag="au", bufs=8,
                                    name=f"au{sb}_{h}")
                    nc.vector.tensor_copy(au[:], po[:])
                    aus[(sb, h)] = au

            def emit_norm_ag(sb):
                rec = opool.tile([128, SB], F32, tag="rec", bufs=2)
                nc.vector.reciprocal(rec[:], sums[sb][:])
                for h in range(QH):
                    rc = opool.tile([1, SB], F32, tag="rc", bufs=2)
                    nc.gpsimd.tensor_copy(rc[:], rec[32 * h:32 * h + 1, :])
                    pb = ps.tile([128, SB], F32, tag="ps", name="pb")
                    nc.tensor.matmul(pb[:], ones_row[:], rc[:],
                                     start=True, stop=True)
                    at = opool.tile([128, SB], BF16, tag="at")
                    nc.vector.tensor_mul(at[:], aus[(sb, h)][:], pb[:])
                    nc.gpsimd.dma_start(aT_loc[sb][bass.ts(h, 128), :], at[:])
                    del aus[(sb, h)]
                nc.gpsimd.collective_compute(
                    "AllGather",
                    mybir.AluOpType.bypass,
                    ins=[aT_loc[sb][:]],
                    outs=[aT_all[sb][:]],
                    replica_groups=[list(range(NCORES))],
                )
                for j in range(4):
                    acol = spool.tile([128, DC, 128], BF16, tag="acol",
                                      bufs=4, name=f"acol{sb}_{j}")
                    nc.gpsimd.dma_start(
                        acol[:],
                        aT_all[sb][:, bass.ts(j, 128)].rearrange(
                            "(c p) m -> p c m", p=128))
                    acols[(sb, j)] = acol

            def outproj_steps(sbp, group=4):
                """Generator: each next() emits `group` out-proj matmuls."""
                for j in range(4):
                    mc = 4 * sbp + j
                    acol = acols.pop((sbp, j))
                    pout = ps.tile([128, SB], F32, tag="ps", name="pout")
                    for hc0 in range(0, DC, group):
                        for hc in range(hc0, min(hc0 + group, DC)):
                            nc.tensor.matmul(pout[:], acol[:, hc, :],
                                             wo_sb[:, hc, :],
                                             start=hc == 0, stop=hc == DC - 1)
                        yield
                    ot = spool.tile([128, SB], F32, tag="ot")
                    nc.vector.tensor_copy(ot[:], pout[:])
                    nc.sync.dma_start(out[bass.ts(mc, 128), :], ot[:])

            def emit_outproj(sbp):
                for _ in outproj_steps(sbp):
                    pass

            # ---- schedule ----
            # PE stream: proj(0) rope(0) attn(0) | proj(1) norm(0)+AG(0)
            # rope(1) attn(1) | proj(2) norm(1)+AG(1) rope(2)+outproj(0)
            # attn(2) | proj(3) norm(2)+AG(2) rope(3)+outproj(1) attn(3) |
            # norm(3)+AG(3) outproj(2) outproj(3)
            def drain(gen):
                for _ in gen:
                    pass

            t0_ = emit_proj(0)
            emit_rope_v(0, t0_)
            emit_attn(0)
            t1_ = emit_proj(1)
            emit_norm_ag(0)
            emit_rope_v(1, t1_)
            emit_attn(1)
            t2_ = emit_proj(2)
            emit_norm_ag(1)
            emit_rope_v(2, t2_)
            go0 = outproj_steps(0, group=3)
            emit_attn(2, go0)         # outproj(0) rides inside attn(2)
            drain(go0)
            t3_ = emit_proj(3)
            emit_norm_ag(2)
            emit_rope_v(3, t3_)
            go1 = outproj_steps(1, group=1)
            emit_attn(3, go1)         # half of outproj(1) rides inside attn(3)
            emit_norm_ag(3)
            drain(go1)                # rest covers the AG(3) window
            emit_outproj(NSB - 2)
            emit_outproj(NSB - 1)

    nc.compile()
    return nc


_PERM = np.concatenate([np.arange(0, HD, 2), np.arange(1, HD, 2)])


def _prep_inputs(x, wq, wk, wv, wo, freqs_cos, freqs_sin):
    xT = np.ascontiguousarray(x.reshape(S, D).T).astype(BF16_NP)
    cosT = np.ascontiguousarray(freqs_cos.T).astype(np.float32)
    sinT = np.ascontiguousarray(freqs_sin.T).astype(np.float32)
    mask = np.triu(np.ones((128, 128), dtype=np.float32)).astype(BF16_NP)

    qperm = np.concatenate([h * HD + _PERM for h in range(QH)])
    in_maps = []
    for c in range(NCORES):
        wq_c = wq[c * QROWS:(c + 1) * QROWS][qperm]
        wk_c = wk[c * HD:(c + 1) * HD][_PERM]
        wv_c = wv[c * HD:(c + 1) * HD]
        wo_c = wo[c * SB:(c + 1) * SB]
        in_maps.append({
            "xT": xT,
            "wqT": np.ascontiguousarray(wq_c.T).astype(BF16_NP),
            "wkT": np.ascontiguousarray(wk_c.T).astype(BF16_NP),
            "wvT": np.ascontiguousarray(wv_c.T).astype(BF16_NP),
            "woT": np.ascontiguousarray(wo_c.T).astype(BF16_NP),
            "cosT": cosT,
            "sinT": sinT,
            "mask": mask,
        })
    return in_maps


def kernel(x, wq, wk, wv, wo, freqs_cos, freqs_sin, start_pos=0, *,
           _trace=False):
    x = np.asarray(x, dtype=np.float32)
    in_maps = _prep_inputs(np.asarray(x, np.float32), np.asarray(wq, np.float32),
                           np.asarray(wk, np.float32), np.asarray(wv, np.float32),
                           np.asarray(wo, np.float32),
                           np.asarray(freqs_cos, np.float32),
                           np.asarray(freqs_sin, np.float32))
    nc = build_graph()
    res = run_bass_kernel_spmd(nc, in_maps, core_ids=list(range(NCORES)),
                               trace=_trace)
    full = np.concatenate([res.results[c]["out"] for c in range(NCORES)],
                          axis=1)
    out = full.reshape(1, S, D).astype(np.float32)
    if _trace:
        return out, res
    return out


# revision 18
# speedup vs baseline: 1.2110x; 1.0058x over previous
"""Multi-head GQA attention prefill (B=1, S=2048, D=4096, 32 q-heads /
8 kv-heads, head_dim=128, RoPE, causal) on 8 TRN2 NeuronCores.

Sharding: tensor-parallel over heads. Core c owns q-heads [4c, 4c+4) and
kv-head c (the GQA group boundary coincides with the core boundary, so
attention is fully local). The out-projection is sharded over wo ROWS
(output columns): after attention each core AllGathers the (transposed,
normalized) attention outputs of all heads and computes its 512 output
columns; the host concatenates the 8 column slices.

Compute dtype: bf16 matmul operands with fp32 PSUM accumulation; softmax
statistics in fp32.  All matmuls run in the transposed "P^T" dataflow:
  qT/kT  [head_dim, S]  = proj(xT)           (RoPE'd in-place on DVE)
  S^T    [Sk, Sq]       = kT_chunk.T @ qT    (causal blocks only)
  expS   bf16           = exp(S^T / sqrt(d)) (ScalarE, PSUM->SBUF)
  oT     [head_dim, Sq] = sum_k V_chunk.T @ expS   (V from PE-transpose)
  rowsum [1, Sq]        = ones.T @ expS      (fp32 via PSUM accumulate)
so no transposes are needed anywhere else in the chain.

RoPE trick: attention scores are invariant under a fixed permutation of
each head's 128 dims applied to BOTH q and k, so the host deinterleaves
wq/wk rows to [evens; odds].  RoPE then becomes two partition-halves
ops: new_e = e*cos - o*sin, new_o = e*sin + o*cos with [64, S] tiles.
"""

import sys

sys.path.insert(0, "/opt/trn_rl_repo")

import numpy as np
import ml_dtypes

import concourse.bass as bass
import concourse.mybir as mybir
from concourse import bacc, tile
from concourse.bass_utils import run_bass_kernel_spmd
from concourse.masks import make_identity

F32 = mybir.dt.float32
BF16 = mybir.dt.bfloat16
BF16_NP = ml_dtypes.bfloat16

NCORES = 8
S = 2048
D = 4096
HD = 128                 # head dim
QH = 4                   # q heads per core
QROWS = QH * HD          # 512 q rows per core
SB = 512                 # seq superblock (free dim of most matmuls)
NSB = S // SB            # 4
DC = D // 128            # 32 contraction chunks
NKC = S // 128           # 16 key chunks
SCALE = 1.0 / np.sqrt(HD)


def build_graph():
    nc = bacc.Bacc("TRN2", target_bir_lowering=False, debug=False,
                   num_devices=NCORES)

    xT = nc.declare_dram_parameter("xT", [D, S], BF16, isOutput=False)
    wqT = nc.declare_dram_parameter("wqT", [D, QROWS], BF16, isOutput=False)
    wkT = nc.declare_dram_parameter("wkT", [D, HD], BF16, isOutput=False)
    wvT = nc.declare_dram_parameter("wvT", [D, HD], BF16, isOutput=False)
    woT = nc.declare_dram_parameter("woT", [D, SB], BF16, isOutput=False)
    cosT = nc.declare_dram_parameter("cosT", [64, S], F32, isOutput=False)
    sinT = nc.declare_dram_parameter("sinT", [64, S], F32, isOutput=False)
    mask = nc.declare_dram_parameter("mask", [128, 128], BF16, isOutput=False)
    out = nc.declare_dram_parameter("out", [S, SB], F32, isOutput=True)

    aT_loc = [nc.dram_tensor(f"aT_loc{sb}", [QROWS, SB], BF16)
              for sb in range(NSB)]
    aT_all = [nc.dram_tensor(f"aT_all{sb}", [NCORES * QROWS, SB], BF16,
                             addr_space="Shared") for sb in range(NSB)]

    with tile.TileContext(nc) as tc:
        with tc.tile_pool(name="const", bufs=1) as cpool, \
             tc.tile_pool(name="wts", bufs=1) as wpool, \
             tc.tile_pool(name="qkv", bufs=1) as qkvpool, \
             tc.tile_pool(name="xs", bufs=4) as xpool, \
             tc.tile_pool(name="rope", bufs=2) as rpool, \
             tc.tile_pool(name="exps", bufs=4) as epool, \
             tc.tile_pool(name="onorm", bufs=2) as opool, \
             tc.tile_pool(name="ostream", bufs=3) as spool, \
             tc.tile_pool(name="ps", bufs=8, space="PSUM") as ps:

            # ---- weights: chunked DMAs so the first matmul starts early ----
            wq_sb = wpool.tile([128, DC, QROWS], BF16, tag="wq")
            wk_sb = wpool.tile([128, DC, HD], BF16, tag="wk")
            wv_sb = wpool.tile([128, DC, HD], BF16, tag="wv")
            wo_sb = wpool.tile([128, DC, SB], BF16, tag="wo")
            wqr = wqT.rearrange("(c p) m -> p c m", p=128)
            wkr = wkT.rearrange("(c p) m -> p c m", p=128)
            wvr = wvT.rearrange("(c p) m -> p c m", p=128)
            wor = woT.rearrange("(c p) m -> p c m", p=128)
            G = 8
            for g in range(0, DC, G):
                s = slice(g, g + G)
                nc.gpsimd.dma_start(wk_sb[:, s, :], wkr[:, s, :])
                nc.gpsimd.dma_start(wv_sb[:, s, :], wvr[:, s, :])
                nc.gpsimd.dma_start(wq_sb[:, s, :], wqr[:, s, :])

            # ---- constants ----
            cos1 = cpool.tile([64, S], F32, tag="cos1")
            nc.gpsimd.dma_start(cos1[:], cosT[:, :])
            sin1 = cpool.tile([64, S], F32, tag="sin1")
            nc.gpsimd.dma_start(sin1[:], sinT[:, :])
            mask_t = cpool.tile([128, 128], BF16, tag="mask")
            nc.gpsimd.dma_start(mask_t[:], mask[:])
            ident = cpool.tile([128, 128], BF16, tag="ident")
            make_identity(nc, ident[:])
            ones_col = cpool.tile([128, 1], F32, tag="ones_col")
            nc.vector.memset(ones_col[:], 1.0)
            ones_row = cpool.tile([1, 128], F32, tag="ones_row")
            nc.vector.memset(ones_row[:], 1.0)
            for g in range(0, DC, G):
                nc.gpsimd.dma_start(wo_sb[:, g:g + G, :], wor[:, g:g + G, :])

            # ---- persistent activations ----
            qT = [qkvpool.tile([128, S], BF16, tag=f"qT{h}", name=f"qT{h}")
                  for h in range(QH)]
            kT = qkvpool.tile([128, S], BF16, tag="kT")
            v_sb = [qkvpool.tile([128, HD], BF16, tag=f"v{kc}", name=f"v{kc}")
                    for kc in range(NKC)]

            aus = {}     # (sb, h) -> unnormalized oT tile
            sums = {}    # sb -> rowsum collection tile
            acols = {}   # (sb, j) -> prefetched gathered-aT column block

            def alloc_proj(sb):
                pq = [ps.tile([128, SB], F32, tag="ps", name=f"pq{sb}_{h}")
                      for h in range(QH)]
                pk = ps.tile([128, SB], F32, tag="ps", name=f"pk{sb}")
                pv = ps.tile([128, SB], F32, tag="ps", name=f"pv{sb}")
                return pq, pk, pv

            def proj_steps(sb, tiles):
                """Two 3-bank passes so a concurrent attention block fits in
                PSUM; yields after every 3 matmuls (one dc of one pass)."""
                pq, pk, pv = tiles
                cols = bass.ts(sb, SB)
                passes = [
                    [(pk, wk_sb, None), (pv, wv_sb, None), (pq[0], wq_sb, 0)],
                    [(pq[1], wq_sb, 1), (pq[2], wq_sb, 2), (pq[3], wq_sb, 3)],
                ]
                for pas in passes:
                    for dc in range(DC):
                        xt = xpool.tile([128, SB], BF16, tag="xt")
                        nc.sync.dma_start(xt[:], xT[bass.ts(dc, 128), cols])
                        st, sp = dc == 0, dc == DC - 1
                        for (pt_, wt, h) in pas:
                            w = wt[:, dc, :] if h is None else \
                                wt[:, dc, bass.ts(h, HD)]
                            nc.tensor.matmul(pt_[:], w, xt[:],
                                             start=st, stop=sp)
                        yield

            def emit_proj(sb, tiles=None):
                tiles = tiles or alloc_proj(sb)
                for _ in proj_steps(sb, tiles):
                    pass
                return tiles

            def rope(psum, dst, cols):
                # dst[e] = p[e]*cos - p[o]*sin ; dst[o] = p[o]*cos + p[e]*sin
                # tb holds the sin-products half-SWAPPED so that the final
                # add/sub reads same-base-partition SBUF operands.
                ta = rpool.tile([128, SB], F32, tag="rope_a")
                tb = rpool.tile([128, SB], F32, tag="rope_b")
                nc.vector.tensor_mul(ta[0:64, :], psum[0:64, :], cos1[:, cols])
                nc.vector.tensor_mul(ta[64:128, :], psum[64:128, :],
                                     cos1[:, cols])
                nc.vector.tensor_mul(tb[0:64, :], psum[64:128, :],
                                     sin1[:, cols])
                nc.vector.tensor_mul(tb[64:128, :], psum[0:64, :],
                                     sin1[:, cols])
                nc.vector.tensor_sub(dst[0:64, cols], ta[0:64, :], tb[0:64, :])
                nc.vector.tensor_add(dst[64:128, cols], ta[64:128, :],
                                     tb[64:128, :])

            def emit_rope_v(sb, tiles):
                pq, pk, pv = tiles
                cols = bass.ts(sb, SB)
                rope(pk, kT, cols)          # k first: attention h=0 needs it
                rope(pq[0], qT[0], cols)
                vt = rpool.tile([128, SB], BF16, tag="vt")
                nc.vector.tensor_copy(vt[:], pv[:])
                for j in range(SB // 128):
                    pt = ps.tile([128, 128], BF16, tag="ps", name="pt")
                    nc.tensor.transpose(pt[:], vt[:, bass.ts(j, 128)], ident[:])
                    nc.vector.tensor_copy(v_sb[4 * sb + j][:], pt[:])
                for h in range(1, QH):
                    rope(pq[h], qT[h], cols)

            def emit_attn(sb, filler=None):
                def fill():
                    if filler is not None:
                        next(filler, None)
                nkc = 4 * sb + 4
                sm = opool.tile([128, SB], F32, tag="sums", bufs=2,
                                name=f"sums{sb}")
                nc.gpsimd.memset(sm[:], 1.0)
                sums[sb] = sm
                for h in range(QH):
                    po = ps.tile([128, SB], F32, tag="ps", name="po")
                    acc = epool.tile([128, SB], F32, tag="acc", bufs=2)
                    es = {}

                    def qk(kc):
                        # diagonal blocks: columns < 128j are fully masked --
                        # skip them in both the matmul and the exp.
                        j = kc - 4 * sb
                        c0 = 128 * j if j > 0 else 0
                        pg = ps.tile([128, SB], F32, tag="ps", name="pg")
                        nc.tensor.matmul(
                            pg[:, c0:SB], kT[:, bass.ts(kc, 128)],
                            qT[h][:, sb * SB + c0:(sb + 1) * SB],
                            start=True, stop=True)
                        e = epool.tile([128, SB], BF16, tag="es")
                        nc.scalar.activation(e[:, c0:SB], pg[:, c0:SB],
                                             mybir.ActivationFunctionType.Exp,
                                             scale=SCALE)
                        if c0 > 0:
                            nc.vector.memset(e[:, 0:c0], 0.0)
                        if j >= 0:
                            nc.vector.tensor_mul(e[:, bass.ts(j, 128)],
                                                 e[:, bass.ts(j, 128)], mask_t[:])
                        es[kc] = e

                    def pv_ones(kc):
                        st, sp = kc == 0, kc == nkc - 1
                        nc.tensor.matmul(po[:], v_sb[kc][:], es[kc][:],
                                         start=st, stop=sp)
                        if st:
                            nc.vector.tensor_copy(acc[:], es[kc][:])
                        else:
                            nc.vector.tensor_add(acc[:], acc[:], es[kc][:])
                        del es[kc]

                    qk(0)
                    for kc in range(1, nkc):
                        qk(kc)
                        pv_ones(kc - 1)
                        fill()
                    pv_ones(nkc - 1)
                    fill()

                    psum = ps.tile([1, SB], F32, tag="ps", name="psum")
                    nc.tensor.matmul(psum[:], ones_col[:], acc[:],
                                     start=True, stop=True)
                    nc.vector.tensor_copy(sm[32 * h:32 * h + 1, :], psum[:])
                    au = opool.tile([128, SB], BF16, tag="au", bufs=8,
                                    name=f"au{sb}_{h}")
                    nc.vector.tensor_copy(au[:], po[:])
                    aus[(sb, h)] = au

            def emit_norm_ag(sb):
                rec = opool.tile([128, SB], F32, tag="rec", bufs=2)
                nc.vector.reciprocal(rec[:], sums[sb][:])
                for h in range(QH):
                    rc = opool.tile([1, SB], F32, tag="rc", bufs=2)
                    nc.gpsimd.tensor_copy(rc[:], rec[32 * h:32 * h + 1, :])
                    pb = ps.tile([128, SB], F32, tag="ps", name="pb")
                    nc.tensor.matmul(pb[:], ones_row[:], rc[:],
                                     start=True, stop=True)
                    at = opool.tile([128, SB], BF16, tag="at")
                    nc.vector.tensor_mul(at[:], aus[(sb, h)][:], pb[:])
                    nc.gpsimd.dma_start(aT_loc[sb][bass.ts(h, 128), :], at[:])
                    del aus[(sb, h)]
                nc.gpsimd.collective_compute(
                    "AllGather",
                    mybir.AluOpType.bypass,
                    ins=[aT_loc[sb][:]],
                    outs=[aT_all[sb][:]],
                    replica_groups=[list(range(NCORES))],
                )
                for j in range(4):
                    acol = spool.tile([128, DC, 128], BF16, tag="acol",
                                      bufs=4, name=f"acol{sb}_{j}")
                    nc.gpsimd.dma_start(
                        acol[:],
                        aT_all[sb][:, bass.ts(j, 128)].rearrange(
                            "(c p) m -> p c m", p=128))
                    acols[(sb, j)] = acol

            def outproj_steps(sbp, group=4):
                """Generator: each next() emits `group` out-proj matmuls."""
                for j in range(4):
                    mc = 4 * sbp + j
                    acol = acols.pop((sbp, j))
                    pout = ps.tile([128, SB], F32, tag="ps", name="pout")
                    for hc0 in range(0, DC, group):
                        for hc in range(hc0, min(hc0 + group, DC)):
                            nc.tensor.matmul(pout[:], acol[:, hc, :],
                                             wo_sb[:, hc, :],
                                             start=hc == 0, stop=hc == DC - 1)
                        yield
                    ot = spool.tile([128, SB], F32, tag="ot")
                    nc.vector.tensor_copy(ot[:], pout[:])
                    nc.sync.dma_start(out[bass.ts(mc, 128), :], ot[:])

            def emit_outproj(sbp):
                for _ in outproj_steps(sbp):
                    pass

            # ---- schedule ----
            # PE stream: proj(0) rope(0) attn(0) | proj(1) norm(0)+AG(0)
            # rope(1) attn(1) | proj(2) norm(1)+AG(1) rope(2)+outproj(0)
            # attn(2) | proj(3) norm(2)+AG(2) rope(3)+outproj(1) attn(3) |
            # norm(3)+AG(3) outproj(2) outproj(3)
            def drain(gen):
                for _ in gen:
                    pass

            t0_ = emit_proj(0)
            emit_rope_v(0, t0_)
            emit_attn(0)
            t1_ = emit_proj(1)
            emit_norm_ag(0)
            emit_rope_v(1, t1_)
            emit_attn(1)
            t2_ = emit_proj(2)
            emit_norm_ag(1)
            emit_rope_v(2, t2_)
            go0 = outproj_steps(0, group=3)
            emit_attn(2, go0)         # outproj(0) rides inside attn(2)
            drain(go0)
            t3_ = emit_proj(3)
            emit_norm_ag(2)
            emit_rope_v(3, t3_)
            go1 = outproj_steps(1, group=3)
            emit_attn(3, go1)         # outproj(1) rides inside attn(3)
            drain(go1)
            emit_norm_ag(3)
            emit_outproj(NSB - 2)
            emit_outproj(NSB - 1)

    nc.compile()
    return nc


_PERM = np.concatenate([np.arange(0, HD, 2), np.arange(1, HD, 2)])


def _prep_inputs(x, wq, wk, wv, wo, freqs_cos, freqs_sin):
    xT = np.ascontiguousarray(x.reshape(S, D).T).astype(BF16_NP)
    cosT = np.ascontiguousarray(freqs_cos.T).astype(np.float32)
    sinT = np.ascontiguousarray(freqs_sin.T).astype(np.float32)
    mask = np.triu(np.ones((128, 128), dtype=np.float32)).astype(BF16_NP)

    qperm = np.concatenate([h * HD + _PERM for h in range(QH)])
    in_maps = []
    for c in range(NCORES):
        wq_c = wq[c * QROWS:(c + 1) * QROWS][qperm]
        wk_c = wk[c * HD:(c + 1) * HD][_PERM]
        wv_c = wv[c * HD:(c + 1) * HD]
        wo_c = wo[c * SB:(c + 1) * SB]
        in_maps.append({
            "xT": xT,
            "wqT": np.ascontiguousarray(wq_c.T).astype(BF16_NP),
            "wkT": np.ascontiguousarray(wk_c.T).astype(BF16_NP),
            "wvT": np.ascontiguousarray(wv_c.T).astype(BF16_NP),
            "woT": np.ascontiguousarray(wo_c.T).astype(BF16_NP),
            "cosT": cosT,
            "sinT": sinT,
            "mask": mask,
        })
    return in_maps


def kernel(x, wq, wk, wv, wo, freqs_cos, freqs_sin, start_pos=0, *,
           _trace=False):
    x = np.asarray(x, dtype=np.float32)
    in_maps = _prep_inputs(np.asarray(x, np.float32), np.asarray(wq, np.float32),
                           np.asarray(wk, np.float32), np.asarray(wv, np.float32),
                           np.asarray(wo, np.float32),
                           np.asarray(freqs_cos, np.float32),
                           np.asarray(freqs_sin, np.float32))
    nc = build_graph()
    res = run_bass_kernel_spmd(nc, in_maps, core_ids=list(range(NCORES)),
                               trace=_trace)
    full = np.concatenate([res.results[c]["out"] for c in range(NCORES)],
                          axis=1)
    out = full.reshape(1, S, D).astype(np.float32)
    if _trace:
        return out, res
    return out


# revision 19
# speedup vs baseline: 1.2116x; 1.0005x over previous
"""Multi-head GQA attention prefill (B=1, S=2048, D=4096, 32 q-heads /
8 kv-heads, head_dim=128, RoPE, causal) on 8 TRN2 NeuronCores.

Sharding: tensor-parallel over heads. Core c owns q-heads [4c, 4c+4) and
kv-head c (the GQA group boundary coincides with the core boundary, so
attention is fully local). The out-projection is sharded over wo ROWS
(output columns): after attention each core AllGathers the (transposed,
normalized) attention outputs of all heads and computes its 512 output
columns; the host concatenates the 8 column slices.

Compute dtype: bf16 matmul operands with fp32 PSUM accumulation; softmax
statistics in fp32.  All matmuls run in the transposed "P^T" dataflow:
  qT/kT  [head_dim, S]  = proj(xT)           (RoPE'd in-place on DVE)
  S^T    [Sk, Sq]       = kT_chunk.T @ qT    (causal blocks only)
  expS   bf16           = exp(S^T / sqrt(d)) (ScalarE, PSUM->SBUF)
  oT     [head_dim, Sq] = sum_k V_chunk.T @ expS   (V from PE-transpose)
  rowsum [1, Sq]        = ones.T @ expS      (fp32 via PSUM accumulate)
so no transposes are needed anywhere else in the chain.

RoPE trick: attention scores are invariant under a fixed permutation of
each head's 128 dims applied to BOTH q and k, so the host deinterleaves
wq/wk rows to [evens; odds].  RoPE then becomes two partition-halves
ops: new_e = e*cos - o*sin, new_o = e*sin + o*cos with [64, S] tiles.
"""

import sys

sys.path.insert(0, "/opt/trn_rl_repo")

import numpy as np
import ml_dtypes

import concourse.bass as bass
import concourse.mybir as mybir
from concourse import bacc, tile
from concourse.bass_utils import run_bass_kernel_spmd
from concourse.masks import make_identity

F32 = mybir.dt.float32
BF16 = mybir.dt.bfloat16
BF16_NP = ml_dtypes.bfloat16

NCORES = 8
S = 2048
D = 4096
HD = 128                 # head dim
QH = 4                   # q heads per core
QROWS = QH * HD          # 512 q rows per core
SB = 512                 # seq superblock (free dim of most matmuls)
NSB = S // SB            # 4
DC = D // 128            # 32 contraction chunks
NKC = S // 128           # 16 key chunks
SCALE = 1.0 / np.sqrt(HD)


def build_graph():
    nc = bacc.Bacc("TRN2", target_bir_lowering=False, debug=False,
                   num_devices=NCORES)

    xT = nc.declare_dram_parameter("xT", [D, S], BF16, isOutput=False)
    wqT = nc.declare_dram_parameter("wqT", [D, QROWS], BF16, isOutput=False)
    wkT = nc.declare_dram_parameter("wkT", [D, HD], BF16, isOutput=False)
    wvT = nc.declare_dram_parameter("wvT", [D, HD], BF16, isOutput=False)
    woT = nc.declare_dram_parameter("woT", [D, SB], BF16, isOutput=False)
    cosT = nc.declare_dram_parameter("cosT", [64, S], F32, isOutput=False)
    sinT = nc.declare_dram_parameter("sinT", [64, S], F32, isOutput=False)
    mask = nc.declare_dram_parameter("mask", [128, 128], BF16, isOutput=False)
    out = nc.declare_dram_parameter("out", [S, SB], F32, isOutput=True)

    aT_loc = [nc.dram_tensor(f"aT_loc{sb}", [QROWS, SB], BF16)
              for sb in range(NSB)]
    aT_all = [nc.dram_tensor(f"aT_all{sb}", [NCORES * QROWS, SB], BF16,
                             addr_space="Shared") for sb in range(NSB)]

    with tile.TileContext(nc) as tc:
        with tc.tile_pool(name="const", bufs=1) as cpool, \
             tc.tile_pool(name="wts", bufs=1) as wpool, \
             tc.tile_pool(name="qkv", bufs=1) as qkvpool, \
             tc.tile_pool(name="xs", bufs=4) as xpool, \
             tc.tile_pool(name="rope", bufs=2) as rpool, \
             tc.tile_pool(name="exps", bufs=4) as epool, \
             tc.tile_pool(name="onorm", bufs=2) as opool, \
             tc.tile_pool(name="ostream", bufs=3) as spool, \
             tc.tile_pool(name="ps", bufs=8, space="PSUM") as ps:

            # ---- weights: chunked DMAs so the first matmul starts early ----
            wq_sb = wpool.tile([128, DC, QROWS], BF16, tag="wq")
            wk_sb = wpool.tile([128, DC, HD], BF16, tag="wk")
            wv_sb = wpool.tile([128, DC, HD], BF16, tag="wv")
            wo_sb = wpool.tile([128, DC, SB], BF16, tag="wo")
            wqr = wqT.rearrange("(c p) m -> p c m", p=128)
            wkr = wkT.rearrange("(c p) m -> p c m", p=128)
            wvr = wvT.rearrange("(c p) m -> p c m", p=128)
            wor = woT.rearrange("(c p) m -> p c m", p=128)
            G = 8
            for g in range(0, DC, G):
                s = slice(g, g + G)
                nc.gpsimd.dma_start(wk_sb[:, s, :], wkr[:, s, :])
                nc.gpsimd.dma_start(wv_sb[:, s, :], wvr[:, s, :])
                nc.gpsimd.dma_start(wq_sb[:, s, :], wqr[:, s, :])

            # ---- constants ----
            cos1 = cpool.tile([64, S], F32, tag="cos1")
            nc.gpsimd.dma_start(cos1[:], cosT[:, :])
            sin1 = cpool.tile([64, S], F32, tag="sin1")
            nc.gpsimd.dma_start(sin1[:], sinT[:, :])
            mask_t = cpool.tile([128, 128], BF16, tag="mask")
            nc.gpsimd.dma_start(mask_t[:], mask[:])
            ident = cpool.tile([128, 128], BF16, tag="ident")
            make_identity(nc, ident[:])
            ones_col = cpool.tile([128, 1], BF16, tag="ones_col")
            nc.vector.memset(ones_col[:], 1.0)
            ones_row = cpool.tile([1, 128], F32, tag="ones_row")
            nc.vector.memset(ones_row[:], 1.0)
            for g in range(0, DC, G):
                nc.gpsimd.dma_start(wo_sb[:, g:g + G, :], wor[:, g:g + G, :])

            # ---- persistent activations ----
            qT = [qkvpool.tile([128, S], BF16, tag=f"qT{h}", name=f"qT{h}")
                  for h in range(QH)]
            kT = qkvpool.tile([128, S], BF16, tag="kT")
            v_sb = [qkvpool.tile([128, HD], BF16, tag=f"v{kc}", name=f"v{kc}")
                    for kc in range(NKC)]

            aus = {}     # (sb, h) -> unnormalized oT tile
            sums = {}    # sb -> rowsum collection tile
            acols = {}   # (sb, j) -> prefetched gathered-aT column block

            def alloc_proj(sb):
                pq = [ps.tile([128, SB], F32, tag="ps", name=f"pq{sb}_{h}")
                      for h in range(QH)]
                pk = ps.tile([128, SB], F32, tag="ps", name=f"pk{sb}")
                pv = ps.tile([128, SB], F32, tag="ps", name=f"pv{sb}")
                return pq, pk, pv

            def proj_steps(sb, tiles):
                """Two 3-bank passes so a concurrent attention block fits in
                PSUM; yields after every 3 matmuls (one dc of one pass)."""
                pq, pk, pv = tiles
                cols = bass.ts(sb, SB)
                passes = [
                    [(pk, wk_sb, None), (pv, wv_sb, None), (pq[0], wq_sb, 0)],
                    [(pq[1], wq_sb, 1), (pq[2], wq_sb, 2), (pq[3], wq_sb, 3)],
                ]
                for pas in passes:
                    for dc in range(DC):
                        xt = xpool.tile([128, SB], BF16, tag="xt")
                        nc.sync.dma_start(xt[:], xT[bass.ts(dc, 128), cols])
                        st, sp = dc == 0, dc == DC - 1
                        for (pt_, wt, h) in pas:
                            w = wt[:, dc, :] if h is None else \
                                wt[:, dc, bass.ts(h, HD)]
                            nc.tensor.matmul(pt_[:], w, xt[:],
                                             start=st, stop=sp)
                        yield

            def emit_proj(sb, tiles=None):
                tiles = tiles or alloc_proj(sb)
                for _ in proj_steps(sb, tiles):
                    pass
                return tiles

            def rope(psum, dst, cols):
                # dst[e] = p[e]*cos - p[o]*sin ; dst[o] = p[o]*cos + p[e]*sin
                # tb holds the sin-products half-SWAPPED so that the final
                # add/sub reads same-base-partition SBUF operands.
                ta = rpool.tile([128, SB], F32, tag="rope_a")
                tb = rpool.tile([128, SB], F32, tag="rope_b")
                nc.vector.tensor_mul(ta[0:64, :], psum[0:64, :], cos1[:, cols])
                nc.vector.tensor_mul(ta[64:128, :], psum[64:128, :],
                                     cos1[:, cols])
                nc.vector.tensor_mul(tb[0:64, :], psum[64:128, :],
                                     sin1[:, cols])
                nc.vector.tensor_mul(tb[64:128, :], psum[0:64, :],
                                     sin1[:, cols])
                nc.vector.tensor_sub(dst[0:64, cols], ta[0:64, :], tb[0:64, :])
                nc.vector.tensor_add(dst[64:128, cols], ta[64:128, :],
                                     tb[64:128, :])

            def emit_rope_v(sb, tiles):
                pq, pk, pv = tiles
                cols = bass.ts(sb, SB)
                rope(pk, kT, cols)          # k first: attention h=0 needs it
                rope(pq[0], qT[0], cols)
                vt = rpool.tile([128, SB], BF16, tag="vt")
                nc.vector.tensor_copy(vt[:], pv[:])
                for j in range(SB // 128):
                    pt = ps.tile([128, 128], BF16, tag="ps", name="pt")
                    nc.tensor.transpose(pt[:], vt[:, bass.ts(j, 128)], ident[:])
                    nc.vector.tensor_copy(v_sb[4 * sb + j][:], pt[:])
                for h in range(1, QH):
                    rope(pq[h], qT[h], cols)

            def emit_attn(sb, filler=None):
                def fill():
                    if filler is not None:
                        next(filler, None)
                nkc = 4 * sb + 4
                sm = opool.tile([128, SB], F32, tag="sums", bufs=2,
                                name=f"sums{sb}")
                nc.gpsimd.memset(sm[:], 1.0)
                sums[sb] = sm
                for h in range(QH):
                    po = ps.tile([128, SB], F32, tag="ps", name="po")
                    psum = ps.tile([1, SB], F32, tag="ps", name="psum")
                    es = {}

                    def qk(kc):
                        # diagonal blocks: columns < 128j are fully masked --
                        # skip them in both the matmul and the exp.
                        j = kc - 4 * sb
                        c0 = 128 * j if j > 0 else 0
                        pg = ps.tile([128, SB], F32, tag="ps", name="pg")
                        nc.tensor.matmul(
                            pg[:, c0:SB], kT[:, bass.ts(kc, 128)],
                            qT[h][:, sb * SB + c0:(sb + 1) * SB],
                            start=True, stop=True)
                        e = epool.tile([128, SB], BF16, tag="es")
                        nc.scalar.activation(e[:, c0:SB], pg[:, c0:SB],
                                             mybir.ActivationFunctionType.Exp,
                                             scale=SCALE)
                        if c0 > 0:
                            nc.vector.memset(e[:, 0:c0], 0.0)
                        if j >= 0:
                            nc.vector.tensor_mul(e[:, bass.ts(j, 128)],
                                                 e[:, bass.ts(j, 128)], mask_t[:])
                        es[kc] = e

                    def pv_ones(kc):
                        st, sp = kc == 0, kc == nkc - 1
                        nc.tensor.matmul(po[:], v_sb[kc][:], es[kc][:],
                                         start=st, stop=sp)
                        nc.tensor.matmul(psum[:], ones_col[:], es[kc][:],
                                         start=st, stop=sp)
                        del es[kc]

                    qk(0)
                    for kc in range(1, nkc):
                        qk(kc)
                        pv_ones(kc - 1)
                        fill()
                    pv_ones(nkc - 1)
                    fill()

                    nc.vector.tensor_copy(sm[32 * h:32 * h + 1, :], psum[:])
                    au = opool.tile([128, SB], BF16, tag="au", bufs=8,
                                    name=f"au{sb}_{h}")
                    nc.vector.tensor_copy(au[:], po[:])
                    aus[(sb, h)] = au

            def emit_norm_ag(sb):
                rec = opool.tile([128, SB], F32, tag="rec", bufs=2)
                nc.vector.reciprocal(rec[:], sums[sb][:])
                for h in range(QH):
                    rc = opool.tile([1, SB], F32, tag="rc", bufs=2)
                    nc.gpsimd.tensor_copy(rc[:], rec[32 * h:32 * h + 1, :])
                    pb = ps.tile([128, SB], F32, tag="ps", name="pb")
                    nc.tensor.matmul(pb[:], ones_row[:], rc[:],
                                     start=True, stop=True)
                    at = opool.tile([128, SB], BF16, tag="at")
                    nc.vector.tensor_mul(at[:], aus[(sb, h)][:], pb[:])
                    nc.gpsimd.dma_start(aT_loc[sb][bass.ts(h, 128), :], at[:])
                    del aus[(sb, h)]
                nc.gpsimd.collective_compute(
                    "AllGather",
                    mybir.AluOpType.bypass,
                    ins=[aT_loc[sb][:]],
                    outs=[aT_all[sb][:]],
                    replica_groups=[list(range(NCORES))],
                )
                for j in range(4):
                    acol = spool.tile([128, DC, 128], BF16, tag="acol",
                                      bufs=4, name=f"acol{sb}_{j}")
                    nc.gpsimd.dma_start(
                        acol[:],
                        aT_all[sb][:, bass.ts(j, 128)].rearrange(
                            "(c p) m -> p c m", p=128))
                    acols[(sb, j)] = acol

            def outproj_steps(sbp, group=4):
                """Generator: each next() emits `group` out-proj matmuls."""
                for j in range(4):
                    mc = 4 * sbp + j
                    acol = acols.pop((sbp, j))
                    pout = ps.tile([128, SB], F32, tag="ps", name="pout")
                    for hc0 in range(0, DC, group):
                        for hc in range(hc0, min(hc0 + group, DC)):
                            nc.tensor.matmul(pout[:], acol[:, hc, :],
                                             wo_sb[:, hc, :],
                                             start=hc == 0, stop=hc == DC - 1)
                        yield
                    ot = spool.tile([128, SB], F32, tag="ot")
                    nc.vector.tensor_copy(ot[:], pout[:])
                    nc.sync.dma_start(out[bass.ts(mc, 128), :], ot[:])

            def emit_outproj(sbp):
                for _ in outproj_steps(sbp):
                    pass

            # ---- schedule ----
            # PE stream: proj(0) rope(0) attn(0) | proj(1) norm(0)+AG(0)
            # rope(1) attn(1) | proj(2) norm(1)+AG(1) rope(2)+outproj(0)
            # attn(2) | proj(3) norm(2)+AG(2) rope(3)+outproj(1) attn(3) |
            # norm(3)+AG(3) outproj(2) outproj(3)
            def drain(gen):
                for _ in gen:
                    pass

            t0_ = emit_proj(0)
            emit_rope_v(0, t0_)
            emit_attn(0)
            t1_ = emit_proj(1)
            emit_norm_ag(0)
            emit_rope_v(1, t1_)
            emit_attn(1)
            t2_ = emit_proj(2)
            emit_norm_ag(1)
            emit_rope_v(2, t2_)
            go0 = outproj_steps(0, group=3)
            emit_attn(2, go0)         # outproj(0) rides inside attn(2)
            drain(go0)
            t3_ = emit_proj(3)
            emit_norm_ag(2)
            emit_rope_v(3, t3_)
            go1 = outproj_steps(1, group=3)
            emit_attn(3, go1)         # outproj(1) rides inside attn(3)
            drain(go1)
            emit_norm_ag(3)
            emit_outproj(NSB - 2)
            emit_outproj(NSB - 1)

    nc.compile()
    return nc


_PERM = np.concatenate([np.arange(0, HD, 2), np.arange(1, HD, 2)])


def _prep_inputs(x, wq, wk, wv, wo, freqs_cos, freqs_sin):
    xT = np.ascontiguousarray(x.reshape(S, D).T).astype(BF16_NP)
    cosT = np.ascontiguousarray(freqs_cos.T).astype(np.float32)
    sinT = np.ascontiguousarray(freqs_sin.T).astype(np.float32)
    mask = np.triu(np.ones((128, 128), dtype=np.float32)).astype(BF16_NP)

    qperm = np.concatenate([h * HD + _PERM for h in range(QH)])
    in_maps = []
    for c in range(NCORES):
        wq_c = wq[c * QROWS:(c + 1) * QROWS][qperm]
        wk_c = wk[c * HD:(c + 1) * HD][_PERM]
        wv_c = wv[c * HD:(c + 1) * HD]
        wo_c = wo[c * SB:(c + 1) * SB]
        in_maps.append({
            "xT": xT,
            "wqT": np.ascontiguousarray(wq_c.T).astype(BF16_NP),
            "wkT": np.ascontiguousarray(wk_c.T).astype(BF16_NP),
            "wvT": np.ascontiguousarray(wv_c.T).astype(BF16_NP),
            "woT": np.ascontiguousarray(wo_c.T).astype(BF16_NP),
            "cosT": cosT,
            "sinT": sinT,
            "mask": mask,
        })
    return in_maps


def kernel(x, wq, wk, wv, wo, freqs_cos, freqs_sin, start_pos=0, *,
           _trace=False):
    x = np.asarray(x, dtype=np.float32)
    in_maps = _prep_inputs(np.asarray(x, np.float32), np.asarray(wq, np.float32),
                           np.asarray(wk, np.float32), np.asarray(wv, np.float32),
                           np.asarray(wo, np.float32),
                           np.asarray(freqs_cos, np.float32),
                           np.asarray(freqs_sin, np.float32))
    nc = build_graph()
    res = run_bass_kernel_spmd(nc, in_maps, core_ids=list(range(NCORES)),
                               trace=_trace)
    full = np.concatenate([res.results[c]["out"] for c in range(NCORES)],
                          axis=1)
    out = full.reshape(1, S, D).astype(np.float32)
    if _trace:
        return out, res
    return out
